# revision 27
# baseline (speedup 1.0000x reference)
"""AlignmentContrastiveLoss (MrSw) TRN2 kernel — packed fp8 DoubleRow einsum.

Data-parallel over images (16 per core).  Two packings cut both PE columns
and HBM traffic:

* im packing: images are sorted by valid-region count il and dealt into 16
  strata (stratum k = rank 8k..8k+7, one image per core).  Stratum k is
  stored at a shared stride_k = max il in the stratum (+1 zero slot when
  il<49 so the reference's masked-0 candidates survive the max).  Packed
  width P = sum(strides) <= 512, so each word needs only 4 accumulating
  K=256 fp8 DoubleRow matmuls into a single PSUM bank.

* s packing: sentences are sorted by valid-word count sl; word w ships only
  its cnt_w = #{j: sl_j > w} valid columns (nested prefixes), so the
  stationary is [dk, 2, cnt_w] and sT shrinks ~40%.  Output partitions
  >= cnt_w are stale; reads are partition-limited and maxr rows are
  re-zeroed per word.

Reductions: ScalarE evacuates PSUM->SBUF bf16 ring; GpSimd+DVE split the
runmax elementwise max; DVE does per-stratum maxr reduces batched in
5-word windows; term2 (sum_r runmax per image) runs on the PE as
transpose + 0/1-mask matmul so the tail stays off DVE.
"""

import numpy as np
import ml_dtypes

import concourse.bacc as bacc
import concourse.mybir as mybir
import concourse.tile as tile
from concourse.bass_utils import run_bass_kernel_spmd

B = 128
L_IM, L_S, D = 50, 40, 1024
R = L_IM - 1     # 49
W = L_S - 3      # 37
NCORES = 8
IPC = B // NCORES            # 16
K4 = D // 256                # 4 double-row contraction chunks
FP8 = mybir.dt.float8e4
BF16 = mybir.dt.bfloat16
F32 = mybir.dt.float32
X = mybir.AxisListType.X
DR = mybir.MatmulPerfMode.DoubleRow

RING = 10        # s-word ring and al ring depth
WIN = 14         # max maxr reduce window (words)
NEG = -1.0e30

_NC_CACHE = {}


def _plan(im_len, s_len):
    il = (np.asarray(im_len).astype(np.int64) - 1)
    sl = (np.asarray(s_len).astype(np.int64) - 3)
    iorder = np.argsort(-il, kind="stable")          # image deal, rank-major
    deal = iorder.reshape(IPC, NCORES)               # [k, c] image ids
    smax = il[deal].max(axis=1)
    strides = np.where(smax < R, smax + 1, R).astype(np.int64)
    # pair adjacent strata (equal stride) while the P<=512 budget lasts so
    # each reduce window needs fewer DVE ops
    budget = 510 - int(strides.sum())   # 512 exactly = pow2 stride pathology
    paired = []
    for k in sorted(range(0, IPC - 1, 2),
                    key=lambda k: strides[k] - strides[k + 1]):
        cost = int(strides[k] - strides[k + 1])
        if cost <= budget:
            budget -= cost
            paired.append(k)
    for k in paired:
        strides[k + 1] = strides[k]
    groups = []                                      # (k0, n_strata, stride)
    k = 0
    while k < IPC:
        if k in paired:
            groups.append((k, 2, int(strides[k])))
            k += 2
        else:
            groups.append((k, 1, int(strides[k])))
            k += 1
    P = int(strides.sum())
    base = np.concatenate([[0], np.cumsum(strides)]).astype(np.int64)
    jorder = np.argsort(-sl, kind="stable")          # sentence perm
    slp = sl[jorder]
    weff = int(slp[0])
    cnt = np.array([(slp > w).sum() for w in range(weff)], dtype=np.int64)
    b = ((cnt + 31) // 32) * 32                      # padded stationary width
    wins = []                                        # (w0, n) within const-b runs
    s = 0
    for w in range(1, weff + 1):
        if w == weff or b[w] != b[s]:
            w0 = s
            while w0 < w:
                n = min(WIN, w - w0)
                wins.append((w0, n))
                w0 += n
            s = w
    if wins[-1][1] > 3:                              # short final window -> short tail
        w0, n = wins.pop()
        wins.extend([(w0, n - 3), (w0 + n - 3, 3)])
    return dict(il=il, sl=sl, deal=deal, strides=strides, P=P, base=base,
                jorder=jorder, weff=weff, cnt=cnt, b=b, wins=wins,
                groups=groups)


def _build(plan):
    strides, base = plan["strides"], plan["base"]
    P, weff, cnt = plan["P"], plan["weff"], plan["cnt"]
    b, wins, groups = plan["b"], plan["wins"], plan["groups"]
    assert P <= 512, P
    soff = np.concatenate([[0], np.cumsum(K4 * 2 * b)]).astype(np.int64)
    pcs = [min(128, P - t * 128) for t in range((P + 127) // 128)]
    npc = len(pcs)

    nc = bacc.Bacc("TRN2", target_bir_lowering=False, debug=False,
                   num_devices=NCORES)
    # sT layout: concat over words of [dk, (k4, h, j<cnt_w)]
    sT = nc.dram_tensor("sT", [128, int(soff[-1])], FP8, kind="ExternalInput")
    # imT layout: [dk, (k4, h, packed_ir)]
    imT = nc.dram_tensor("imT", [128, K4 * 2 * P], FP8, kind="ExternalInput")
    # mk layout: [p, (pchunk, i)] 0/1 stratum-membership mask
    mk = nc.dram_tensor("mk", [128, npc * IPC], BF16, kind="ExternalInput")
    eye = nc.dram_tensor("eye", [128, 128], BF16, kind="ExternalInput")
    out = nc.dram_tensor("out", [B, IPC], F32, kind="ExternalOutput")

    with tile.TileContext(nc) as tc:
        with (
            tc.tile_pool(name="persist", bufs=1) as persist,
            tc.tile_pool(name="ps", bufs=5, space="PSUM") as pspool,
        ):
            # --- startup DMAs -------------------------------------------
            # ACT table pre-load first so it is done before the w=0 copy
            dummy = persist.tile([128, 128], BF16, name="dummy")
            nc.vector.memset(dummy[:], 0)
            trash = persist.tile([128, 4], BF16, name="trash")
            nc.scalar.copy(trash[:, 0:1], dummy[:, 0:1])

            imt = [persist.tile([128, 2 * P], FP8, name=f"imt{c}")
                   for c in range(K4)]
            for c in range(K4):
                nc.scalar.dma_start(imt[c][:],
                                    imT.ap()[:, c * 2 * P:(c + 1) * 2 * P])

            # all s words in one persistent tile; per-word DMAs paced with
            # ~10 words of lookahead so transfers never burst (SBUF-port
            # contention with the PE) nor starve the stream
            s_all = persist.tile([128, int(soff[-1])], FP8, name="s_all")

            def s_dma(w):
                nc.sync.dma_start(s_all[:, int(soff[w]):int(soff[w + 1])],
                                  sT.ap()[:, int(soff[w]):int(soff[w + 1])])

            for w in range(min(RING, weff)):
                s_dma(w)
            mkt = persist.tile([128, npc * IPC], BF16, name="mkt")
            nc.scalar.dma_start(mkt[:], mk.ap()[:])
            eyet = persist.tile([128, 128], BF16, name="eyet")
            nc.scalar.dma_start(eyet[:], eye.ap()[:])

            def s_lhsT(w):      # [128, 2, b_w] zero-padded stationary
                return s_all[:, int(soff[w]):int(soff[w + 1])].rearrange(
                    "p (c h j) -> p c h j", c=K4, h=2)

            def im_c(c):        # [128, 2, P] moving operand
                return imt[c][:].rearrange("p (h n) -> p h n", h=2)

            # --- PE warm-up (clock ramp) while DMAs stream --------------
            warm = pspool.tile([128, 512], F32, tag="ps", name="warm")
            for _ in range(20):
                nc.tensor.matmul(warm[:, 0:128], dummy[:], dummy[:],
                                 start=True, stop=True)

            # --- persistent state ---------------------------------------
            # one al tile per reduce window: an Act copy's WAR then only
            # couples to its own window's readers (whole-tile fallback deps
            # otherwise serialize copies behind unrelated reduce bursts)
            alw = {}
            for (w0, n) in wins:
                alw[w0] = persist.tile([128, n, P], BF16, name=f"al{w0}")
            w2win = {}
            for (w0, n) in wins:
                for v in range(w0, w0 + n):
                    w2win[v] = (w0, v - w0)
            runmax = persist.tile([128, P], BF16, name="runmax")
            maxr = persist.tile([128, IPC, weff], BF16, name="maxr")
            psT = pspool.tile([128, npc * 128], BF16, tag="psT", name="psT", bufs=1)
            nc.gpsimd.memset(runmax[:], 0)
            nc.gpsimd.memset(maxr[:], 0)
            # PSUM can't take a bf16 memset; zero it through an f32 view
            # (gpsimd can't touch PSUM, so this one stays on DVE)
            nc.vector.memset(psT[:].bitcast(F32), 0)

            win_end = {w0 + n - 1: (w0, n) for (w0, n) in wins}
            for w in range(weff):
                bw = int(b[w])
                ps = pspool.tile([128, 512], F32, tag="ps", name=f"ps{w}")
                lhsT = s_lhsT(w)
                for c in range(K4):
                    nc.tensor.matmul(ps[0:bw, 0:P], lhsT[:, c, :, :],
                                     im_c(c), start=(c == 0),
                                     stop=(c == K4 - 1), perf_mode=DR)
                a0, ai = w2win[w]
                nc.scalar.copy(alw[a0][0:bw, ai, :], ps[0:bw, 0:P])
                nc.vector.tensor_max(runmax[0:bw, :], runmax[0:bw, :],
                                     alw[a0][0:bw, ai, :])
                if w + RING < weff:
                    s_dma(w + RING)
                # window maxr reduces (rows >= b are untouched zeros)
                if w in win_end:
                    w0, n = win_end[w]
                    at = alw[w0]
                    for (k0, g, s) in groups:
                        b0 = int(base[k0])
                        if g == 1:
                            nc.vector.reduce_max(
                                maxr[0:bw, k0, w0:w0 + n],
                                at[0:bw, 0:n, b0:b0 + s], axis=X)
                        else:
                            # g (stratum) outer so the out keeps w innermost
                            # at stride 1 (2x_1p needs a packed last dim)
                            nc.vector.reduce_max(
                                maxr[0:bw, k0:k0 + g, w0:w0 + n],
                                at[0:bw, 0:n, b0:b0 + g * s].rearrange(
                                    "p w (g s) -> p g w s", g=g),
                                axis=X)

            # --- tail ----------------------------------------------------
            # term1[j, i] = sum_w maxr
            t1 = persist.tile([128, IPC], F32, name="t1")
            nc.vector.reduce_sum(t1[:], maxr[:], axis=X)
            # term2[i, j] = sum_r runmax via transpose + 0/1 mask matmul
            pv = psT[:].rearrange("p (t j) -> p t j", t=npc)
            for t, pc in enumerate(pcs):
                nc.tensor.transpose(pv[0:pc, t, :],
                                    runmax[:, t * 128:t * 128 + pc], eyet[:])
            rmT = persist.tile([128, npc, 128], BF16, name="rmT")
            nc.scalar.copy(rmT[:], pv)
            ps2 = pspool.tile([IPC, 128], F32, tag="ps2", name="ps2", bufs=1)
            for t, pc in enumerate(pcs):
                nc.tensor.matmul(ps2[:], mkt[0:pc, t * IPC:(t + 1) * IPC],
                                 rmT[0:pc, t, :], start=(t == 0),
                                 stop=(t == npc - 1))
            # transpose term2 back to [j, i] (bf16) and add to term1
            t2sb = persist.tile([IPC, B], BF16, name="t2sb")
            nc.scalar.copy(t2sb[:], ps2[:])
            psT2 = pspool.tile([B, IPC], BF16, tag="psT2", name="psT2", bufs=1)
            nc.tensor.transpose(psT2[:], t2sb[:], eyet[0:IPC, 0:IPC])
            res = persist.tile([B, IPC], F32, name="res")
            nc.vector.tensor_add(res[:], t1[:], psT2[:])
            nc.sync.dma_start(out.ap()[:], res[:])

    nc.compile()
    return nc


def _get_nc(plan):
    key = (tuple(plan["strides"].tolist()), tuple(plan["cnt"].tolist()))
    if key not in _NC_CACHE:
        _NC_CACHE[key] = _build(plan)
    return _NC_CACHE[key]


def kernel(im_set, s_seq, im_len, s_len):
    im_set = np.asarray(im_set, dtype=np.float32)
    s_seq = np.asarray(s_seq, dtype=np.float32)
    plan = _plan(im_len, s_len)
    il, deal = plan["il"], plan["deal"]
    strides, P, base = plan["strides"], plan["P"], plan["base"]
    jorder, weff, cnt = plan["jorder"], plan["weff"], plan["cnt"]

    im = im_set[:, 1:, :]                    # [B, R, D]
    s = s_seq[:, 1:-2, :]                    # [B, W, D]

    # sT: concat over words of [dk, (k4, h, j<b_w)], zero-padded to b_w
    bws = plan["b"]
    s_j = s[jorder]                          # [B sorted, W, D]
    parts = []
    for w in range(weff):
        cw, bw = int(cnt[w]), int(bws[w])
        blk = np.zeros((D, bw), dtype=np.float32)
        blk[:, :cw] = s_j[:cw, w, :].T
        parts.append(blk.reshape(K4, 2, 128, bw)
                     .transpose(2, 0, 1, 3).reshape(128, K4 * 2 * bw))
    sT = np.ascontiguousarray(np.concatenate(parts, axis=1)).astype(
        ml_dtypes.float8_e4m3)

    # mk: 0/1 stratum-membership mask [p, (pchunk, i)]
    npc = (P + 127) // 128
    mkm = np.zeros((128, npc, IPC), dtype=np.float32)
    for k in range(IPC):
        for g in range(int(base[k]), int(base[k + 1])):
            mkm[g % 128, g // 128, k] = 1.0
    mkm = mkm.reshape(128, npc * IPC).astype(ml_dtypes.bfloat16)
    eye = np.eye(128, dtype=np.float32).astype(ml_dtypes.bfloat16)

    in_maps = []
    for c in range(NCORES):
        imp = np.zeros((P, D), dtype=np.float32)
        for k in range(IPC):
            img = deal[k, c]
            n = int(il[img])
            imp[int(base[k]):int(base[k]) + n] = im[img, :n]
        imT = (imp.T.reshape(K4, 2, 128, P).transpose(2, 0, 1, 3)
               .reshape(128, K4 * 2 * P).astype(ml_dtypes.float8_e4m3))
        in_maps.append({"sT": sT, "imT": np.ascontiguousarray(imT),
                        "mk": mkm, "eye": eye})

    nc = _get_nc(plan)
    # The accelerator sporadically reports NRT_EXEC_UNIT_UNRECOVERABLE on the
    # first execution of a freshly loaded NEFF; it recovers after a pause.
    import time
    res = None
    for attempt in range(4):
        try:
            res = run_bass_kernel_spmd(nc, in_maps,
                                       core_ids=list(range(NCORES)))
            break
        except Exception:
            if attempt == 3:
                raise
            time.sleep(15 * (attempt + 1))

    full = np.empty((B, B), dtype=np.float32)
    for c in range(NCORES):
        o = res.results[c]["out"]            # [B, IPC] rows=sorted j, cols=strata
        full[deal[:, c][:, None], jorder[None, :]] = o.T
    return full


# revision 28
# speedup vs baseline: 1.1688x; 1.1688x over previous
"""AlignmentContrastiveLoss (MrSw) TRN2 kernel — packed fp8 DoubleRow einsum.

Data-parallel over images (16 per core).  Two packings cut both PE columns
and HBM traffic:

* im packing: images are sorted by valid-region count il and dealt into 16
  strata (stratum k = rank 8k..8k+7, one image per core).  Stratum k is
  stored at a shared stride_k = max il in the stratum (+1 zero slot when
  il<49 so the reference's masked-0 candidates survive the max).  Packed
  width P = sum(strides) <= 512, so each word needs only 4 accumulating
  K=256 fp8 DoubleRow matmuls into a single PSUM bank.

* s packing: sentences are sorted by valid-word count sl; word w ships only
  its cnt_w = #{j: sl_j > w} valid columns (nested prefixes), so the
  stationary is [dk, 2, cnt_w] and sT shrinks ~40%.  Output partitions
  >= cnt_w are stale; reads are partition-limited and maxr rows are
  re-zeroed per word.

Reductions: ScalarE evacuates PSUM->SBUF bf16 ring; GpSimd+DVE split the
runmax elementwise max; DVE does per-stratum maxr reduces batched in
5-word windows; term2 (sum_r runmax per image) runs on the PE as
transpose + 0/1-mask matmul so the tail stays off DVE.
"""

import numpy as np
import ml_dtypes

import concourse.bacc as bacc
import concourse.mybir as mybir
import concourse.tile as tile
from concourse.bass_utils import run_bass_kernel_spmd

B = 128
L_IM, L_S, D = 50, 40, 1024
R = L_IM - 1     # 49
W = L_S - 3      # 37
NCORES = 8
IPC = B // NCORES            # 16
K4 = D // 256                # 4 double-row contraction chunks
FP8 = mybir.dt.float8e4
BF16 = mybir.dt.bfloat16
F16 = mybir.dt.float16
F32 = mybir.dt.float32
X = mybir.AxisListType.X
DR = mybir.MatmulPerfMode.DoubleRow

RING = 10        # s-word ring and al ring depth
WIN = 14         # max maxr reduce window (words)
NEG = -1.0e30

_NC_CACHE = {}


def _plan(im_len, s_len):
    il = (np.asarray(im_len).astype(np.int64) - 1)
    sl = (np.asarray(s_len).astype(np.int64) - 3)
    iorder = np.argsort(-il, kind="stable")          # image deal, rank-major
    deal = iorder.reshape(IPC, NCORES)               # [k, c] image ids
    smax = il[deal].max(axis=1)
    strides = np.where(smax < R, smax + 1, R).astype(np.int64)
    # pair adjacent strata (equal stride) while the P<=512 budget lasts so
    # each reduce window needs fewer DVE ops
    budget = 510 - int(strides.sum())   # 512 exactly = pow2 stride pathology
    paired = []
    for k in sorted(range(0, IPC - 1, 2),
                    key=lambda k: strides[k] - strides[k + 1]):
        cost = int(strides[k] - strides[k + 1])
        if cost <= budget:
            budget -= cost
            paired.append(k)
    for k in paired:
        strides[k + 1] = strides[k]
    groups = []                                      # (k0, n_strata, stride)
    k = 0
    while k < IPC:
        if k in paired:
            groups.append((k, 2, int(strides[k])))
            k += 2
        else:
            groups.append((k, 1, int(strides[k])))
            k += 1
    P = int(strides.sum())
    base = np.concatenate([[0], np.cumsum(strides)]).astype(np.int64)
    jorder = np.argsort(-sl, kind="stable")          # sentence perm
    slp = sl[jorder]
    weff = int(slp[0])
    cnt = np.array([(slp > w).sum() for w in range(weff)], dtype=np.int64)
    b = ((cnt + 31) // 32) * 32                      # padded stationary width
    wins = []                                        # (w0, n) within const-b runs
    s = 0
    for w in range(1, weff + 1):
        if w == weff or b[w] != b[s]:
            w0 = s
            while w0 < w:
                n = min(WIN, w - w0)
                wins.append((w0, n))
                w0 += n
            s = w
    if wins[-1][1] > 3:                              # short final window -> short tail
        w0, n = wins.pop()
        wins.extend([(w0, n - 3), (w0 + n - 3, 3)])
    return dict(il=il, sl=sl, deal=deal, strides=strides, P=P, base=base,
                jorder=jorder, weff=weff, cnt=cnt, b=b, wins=wins,
                groups=groups)


def _build(plan):
    strides, base = plan["strides"], plan["base"]
    P, weff, cnt = plan["P"], plan["weff"], plan["cnt"]
    b, wins, groups = plan["b"], plan["wins"], plan["groups"]
    assert P <= 512, P
    soff = np.concatenate([[0], np.cumsum(K4 * 2 * b)]).astype(np.int64)
    pcs = [min(128, P - t * 128) for t in range((P + 127) // 128)]
    npc = len(pcs)

    nc = bacc.Bacc("TRN2", target_bir_lowering=False, debug=False,
                   num_devices=NCORES)
    # sT layout: concat over words of [dk, (k4, h, j<cnt_w)]
    sT = nc.dram_tensor("sT", [128, int(soff[-1])], FP8, kind="ExternalInput")
    # imT layout: [dk, (k4, h, packed_ir)]
    imT = nc.dram_tensor("imT", [128, K4 * 2 * P], FP8, kind="ExternalInput")
    # mk layout: [p, (pchunk, i)] 0/1 stratum-membership mask
    mk = nc.dram_tensor("mk", [128, npc * IPC], F16, kind="ExternalInput")
    eye = nc.dram_tensor("eye", [128, 128], F16, kind="ExternalInput")
    out = nc.dram_tensor("out", [B, IPC], F32, kind="ExternalOutput")

    with tile.TileContext(nc) as tc:
        with (
            tc.tile_pool(name="persist", bufs=1) as persist,
            tc.tile_pool(name="ps", bufs=5, space="PSUM") as pspool,
        ):
            # --- startup DMAs -------------------------------------------
            # ACT table pre-load first so it is done before the w=0 copy
            dummy = persist.tile([128, 128], BF16, name="dummy")
            nc.vector.memset(dummy[:], 0)
            trash = persist.tile([128, 4], BF16, name="trash")
            nc.scalar.copy(trash[:, 0:1], dummy[:, 0:1])

            imt = [persist.tile([128, 2 * P], FP8, name=f"imt{c}")
                   for c in range(K4)]
            for c in range(K4):
                nc.scalar.dma_start(imt[c][:],
                                    imT.ap()[:, c * 2 * P:(c + 1) * 2 * P])

            # all s words in one persistent tile; per-word DMAs paced with
            # ~10 words of lookahead so transfers never burst (SBUF-port
            # contention with the PE) nor starve the stream
            s_all = persist.tile([128, int(soff[-1])], FP8, name="s_all")

            def s_dma(w):
                nc.sync.dma_start(s_all[:, int(soff[w]):int(soff[w + 1])],
                                  sT.ap()[:, int(soff[w]):int(soff[w + 1])])

            for w in range(min(RING, weff)):
                s_dma(w)
            mkt = persist.tile([128, npc * IPC], F16, name="mkt")
            nc.scalar.dma_start(mkt[:], mk.ap()[:])
            eyet = persist.tile([128, 128], F16, name="eyet")
            nc.scalar.dma_start(eyet[:], eye.ap()[:])

            def s_lhsT(w):      # [128, 2, b_w] zero-padded stationary
                return s_all[:, int(soff[w]):int(soff[w + 1])].rearrange(
                    "p (c h j) -> p c h j", c=K4, h=2)

            def im_c(c):        # [128, 2, P] moving operand
                return imt[c][:].rearrange("p (h n) -> p h n", h=2)

            # --- PE warm-up (clock ramp) while DMAs stream --------------
            warm = pspool.tile([128, 512], F32, tag="ps", name="warm")
            for _ in range(20):
                nc.tensor.matmul(warm[:, 0:128], dummy[:], dummy[:],
                                 start=True, stop=True)

            # --- persistent state ---------------------------------------
            # one al tile per reduce window: an Act copy's WAR then only
            # couples to its own window's readers (whole-tile fallback deps
            # otherwise serialize copies behind unrelated reduce bursts)
            alw = {}
            for (w0, n) in wins:
                alw[w0] = persist.tile([128, n, P], F16, name=f"al{w0}")
            w2win = {}
            for (w0, n) in wins:
                for v in range(w0, w0 + n):
                    w2win[v] = (w0, v - w0)
            runmax = persist.tile([128, P], F16, name="runmax")
            maxr = persist.tile([128, IPC, weff], F16, name="maxr")
            psT = pspool.tile([128, npc * 128], F16, tag="psT", name="psT", bufs=1)
            nc.gpsimd.memset(runmax[:], 0)
            nc.gpsimd.memset(maxr[:], 0)
            # PSUM can't take a bf16 memset; zero it through an f32 view
            # (gpsimd can't touch PSUM, so this one stays on DVE)
            nc.vector.memset(psT[:].bitcast(F32), 0)

            win_end = {w0 + n - 1: (w0, n) for (w0, n) in wins}
            for w in range(weff):
                bw = int(b[w])
                ps = pspool.tile([128, 512], F32, tag="ps", name=f"ps{w}")
                lhsT = s_lhsT(w)
                for c in range(K4):
                    nc.tensor.matmul(ps[0:bw, 0:P], lhsT[:, c, :, :],
                                     im_c(c), start=(c == 0),
                                     stop=(c == K4 - 1), perf_mode=DR)
                a0, ai = w2win[w]
                nc.scalar.copy(alw[a0][0:bw, ai, :], ps[0:bw, 0:P])
                nc.vector.tensor_max(runmax[0:bw, :], runmax[0:bw, :],
                                     alw[a0][0:bw, ai, :])
                if w + RING < weff:
                    s_dma(w + RING)
                # window maxr reduces (rows >= b are untouched zeros)
                if w in win_end:
                    w0, n = win_end[w]
                    at = alw[w0]
                    for (k0, g, s) in groups:
                        b0 = int(base[k0])
                        if g == 1:
                            nc.vector.reduce_max(
                                maxr[0:bw, k0, w0:w0 + n],
                                at[0:bw, 0:n, b0:b0 + s], axis=X)
                        else:
                            # g (stratum) outer so the out keeps w innermost
                            # at stride 1 (2x_1p needs a packed last dim)
                            nc.vector.reduce_max(
                                maxr[0:bw, k0:k0 + g, w0:w0 + n],
                                at[0:bw, 0:n, b0:b0 + g * s].rearrange(
                                    "p w (g s) -> p g w s", g=g),
                                axis=X)

            # --- tail ----------------------------------------------------
            # term1[j, i] = sum_w maxr
            t1 = persist.tile([128, IPC], F32, name="t1")
            nc.vector.reduce_sum(t1[:], maxr[:], axis=X)
            # term2[i, j] = sum_r runmax via transpose + 0/1 mask matmul
            pv = psT[:].rearrange("p (t j) -> p t j", t=npc)
            for t, pc in enumerate(pcs):
                nc.tensor.transpose(pv[0:pc, t, :],
                                    runmax[:, t * 128:t * 128 + pc], eyet[:])
            rmT = persist.tile([128, npc, 128], F16, name="rmT")
            nc.scalar.copy(rmT[:], pv)
            ps2 = pspool.tile([IPC, 128], F32, tag="ps2", name="ps2", bufs=1)
            for t, pc in enumerate(pcs):
                nc.tensor.matmul(ps2[:], mkt[0:pc, t * IPC:(t + 1) * IPC],
                                 rmT[0:pc, t, :], start=(t == 0),
                                 stop=(t == npc - 1))
            # transpose term2 back to [j, i] (bf16) and add to term1
            t2sb = persist.tile([IPC, B], F16, name="t2sb")
            nc.scalar.copy(t2sb[:], ps2[:])
            psT2 = pspool.tile([B, IPC], F16, tag="psT2", name="psT2", bufs=1)
            nc.tensor.transpose(psT2[:], t2sb[:], eyet[0:IPC, 0:IPC])
            res = persist.tile([B, IPC], F32, name="res")
            nc.vector.tensor_add(res[:], t1[:], psT2[:])
            nc.sync.dma_start(out.ap()[:], res[:])

    nc.compile()
    return nc


def _get_nc(plan):
    key = (tuple(plan["strides"].tolist()), tuple(plan["cnt"].tolist()))
    if key not in _NC_CACHE:
        _NC_CACHE[key] = _build(plan)
    return _NC_CACHE[key]


def kernel(im_set, s_seq, im_len, s_len):
    im_set = np.asarray(im_set, dtype=np.float32)
    s_seq = np.asarray(s_seq, dtype=np.float32)
    plan = _plan(im_len, s_len)
    il, deal = plan["il"], plan["deal"]
    strides, P, base = plan["strides"], plan["P"], plan["base"]
    jorder, weff, cnt = plan["jorder"], plan["weff"], plan["cnt"]

    im = im_set[:, 1:, :]                    # [B, R, D]
    s = s_seq[:, 1:-2, :]                    # [B, W, D]

    # sT: concat over words of [dk, (k4, h, j<b_w)], zero-padded to b_w
    bws = plan["b"]
    s_j = s[jorder]                          # [B sorted, W, D]
    parts = []
    for w in range(weff):
        cw, bw = int(cnt[w]), int(bws[w])
        blk = np.zeros((D, bw), dtype=np.float32)
        blk[:, :cw] = s_j[:cw, w, :].T
        parts.append(blk.reshape(K4, 2, 128, bw)
                     .transpose(2, 0, 1, 3).reshape(128, K4 * 2 * bw))
    sT = np.ascontiguousarray(np.concatenate(parts, axis=1)).astype(
        ml_dtypes.float8_e4m3)

    # mk: 0/1 stratum-membership mask [p, (pchunk, i)]
    npc = (P + 127) // 128
    mkm = np.zeros((128, npc, IPC), dtype=np.float32)
    for k in range(IPC):
        for g in range(int(base[k]), int(base[k + 1])):
            mkm[g % 128, g // 128, k] = 1.0
    mkm = mkm.reshape(128, npc * IPC).astype(np.float16)
    eye = np.eye(128, dtype=np.float32).astype(np.float16)

    in_maps = []
    for c in range(NCORES):
        imp = np.zeros((P, D), dtype=np.float32)
        for k in range(IPC):
            img = deal[k, c]
            n = int(il[img])
            imp[int(base[k]):int(base[k]) + n] = im[img, :n]
        imT = (imp.T.reshape(K4, 2, 128, P).transpose(2, 0, 1, 3)
               .reshape(128, K4 * 2 * P).astype(ml_dtypes.float8_e4m3))
        in_maps.append({"sT": sT, "imT": np.ascontiguousarray(imT),
                        "mk": mkm, "eye": eye})

    nc = _get_nc(plan)
    # The accelerator sporadically reports NRT_EXEC_UNIT_UNRECOVERABLE on the
    # first execution of a freshly loaded NEFF; it recovers after a pause.
    import time
    res = None
    for attempt in range(4):
        try:
            res = run_bass_kernel_spmd(nc, in_maps,
                                       core_ids=list(range(NCORES)))
            break
        except Exception:
            if attempt == 3:
                raise
            time.sleep(15 * (attempt + 1))

    full = np.empty((B, B), dtype=np.float32)
    for c in range(NCORES):
        o = res.results[c]["out"]            # [B, IPC] rows=sorted j, cols=strata
        full[deal[:, c][:, None], jorder[None, :]] = o.T
    return full


# revision 29
# speedup vs baseline: 1.2130x; 1.0378x over previous
"""AlignmentContrastiveLoss (MrSw) TRN2 kernel — packed fp8 DoubleRow einsum.

Data-parallel over images (16 per core).  Two packings cut both PE columns
and HBM traffic:

* im packing: images are sorted by valid-region count il and dealt into 16
  strata (stratum k = rank 8k..8k+7, one image per core).  Stratum k is
  stored at a shared stride_k = max il in the stratum (+1 zero slot when
  il<49 so the reference's masked-0 candidates survive the max).  Packed
  width P = sum(strides) <= 512, so each word needs only 4 accumulating
  K=256 fp8 DoubleRow matmuls into a single PSUM bank.

* s packing: sentences are sorted by valid-word count sl; word w ships only
  its cnt_w = #{j: sl_j > w} valid columns (nested prefixes), so the
  stationary is [dk, 2, cnt_w] and sT shrinks ~40%.  Output partitions
  >= cnt_w are stale; reads are partition-limited and maxr rows are
  re-zeroed per word.

Reductions: ScalarE evacuates PSUM->SBUF bf16 ring; GpSimd+DVE split the
runmax elementwise max; DVE does per-stratum maxr reduces batched in
5-word windows; term2 (sum_r runmax per image) runs on the PE as
transpose + 0/1-mask matmul so the tail stays off DVE.
"""

import numpy as np
import ml_dtypes

import concourse.bacc as bacc
import concourse.mybir as mybir
import concourse.tile as tile
from concourse.bass_utils import run_bass_kernel_spmd

B = 128
L_IM, L_S, D = 50, 40, 1024
R = L_IM - 1     # 49
W = L_S - 3      # 37
NCORES = 8
IPC = B // NCORES            # 16
K4 = D // 256                # 4 double-row contraction chunks
FP8 = mybir.dt.float8e4
BF16 = mybir.dt.bfloat16
F16 = mybir.dt.float16
F32 = mybir.dt.float32
X = mybir.AxisListType.X
DR = mybir.MatmulPerfMode.DoubleRow

RING = 10        # s-word ring and al ring depth
WIN = 14         # max maxr reduce window (words)
NEG = -1.0e30

_NC_CACHE = {}


def _plan(im_len, s_len):
    il = (np.asarray(im_len).astype(np.int64) - 1)
    sl = (np.asarray(s_len).astype(np.int64) - 3)
    iorder = np.argsort(-il, kind="stable")          # image deal, rank-major
    deal = iorder.reshape(IPC, NCORES)               # [k, c] image ids
    smax = il[deal].max(axis=1)
    strides = np.where(smax < R, smax + 1, R).astype(np.int64)
    # pair adjacent strata (equal stride) while the P<=512 budget lasts so
    # each reduce window needs fewer DVE ops
    budget = 510 - int(strides.sum())   # 512 exactly = pow2 stride pathology
    paired = []
    for k in sorted(range(0, IPC - 1, 2),
                    key=lambda k: strides[k] - strides[k + 1]):
        cost = int(strides[k] - strides[k + 1])
        if cost <= budget:
            budget -= cost
            paired.append(k)
    for k in paired:
        strides[k + 1] = strides[k]
    groups = []                                      # (k0, n_strata, stride)
    k = 0
    while k < IPC:
        if k in paired:
            groups.append((k, 2, int(strides[k])))
            k += 2
        else:
            groups.append((k, 1, int(strides[k])))
            k += 1
    P = int(strides.sum())
    base = np.concatenate([[0], np.cumsum(strides)]).astype(np.int64)
    jorder = np.argsort(-sl, kind="stable")          # sentence perm
    slp = sl[jorder]
    weff = int(slp[0])
    cnt = np.array([(slp > w).sum() for w in range(weff)], dtype=np.int64)
    b = ((cnt + 31) // 32) * 32                      # padded stationary width
    wins = []                                        # (w0, n) within const-b runs
    s = 0
    for w in range(1, weff + 1):
        if w == weff or b[w] != b[s]:
            w0 = s
            while w0 < w:
                n = min(WIN, w - w0)
                wins.append((w0, n))
                w0 += n
            s = w
    if wins[-1][1] > 3:                              # short final window -> short tail
        w0, n = wins.pop()
        wins.extend([(w0, n - 3), (w0 + n - 3, 3)])
    return dict(il=il, sl=sl, deal=deal, strides=strides, P=P, base=base,
                jorder=jorder, weff=weff, cnt=cnt, b=b, wins=wins,
                groups=groups)


def _build(plan):
    strides, base = plan["strides"], plan["base"]
    P, weff, cnt = plan["P"], plan["weff"], plan["cnt"]
    b, wins, groups = plan["b"], plan["wins"], plan["groups"]
    assert P <= 512, P
    soff = np.concatenate([[0], np.cumsum(K4 * 2 * b)]).astype(np.int64)
    pcs = [min(128, P - t * 128) for t in range((P + 127) // 128)]
    npc = len(pcs)

    nc = bacc.Bacc("TRN2", target_bir_lowering=False, debug=False,
                   num_devices=NCORES)
    # sT layout: concat over words of [dk, (k4, h, j<cnt_w)]
    sT = nc.dram_tensor("sT", [128, int(soff[-1])], FP8, kind="ExternalInput")
    # imT layout: [dk, (k4, h, packed_ir)]
    imT = nc.dram_tensor("imT", [128, K4 * 2 * P], FP8, kind="ExternalInput")
    # mk layout: [p, (pchunk, i)] 0/1 stratum-membership mask
    mk = nc.dram_tensor("mk", [128, npc * IPC], F16, kind="ExternalInput")
    eye = nc.dram_tensor("eye", [128, 128], F16, kind="ExternalInput")
    out = nc.dram_tensor("out", [B, IPC], F32, kind="ExternalOutput")

    with tile.TileContext(nc) as tc:
        with (
            tc.tile_pool(name="persist", bufs=1) as persist,
            tc.tile_pool(name="ps", bufs=5, space="PSUM") as pspool,
        ):
            # --- startup DMAs -------------------------------------------
            # ACT table pre-load first so it is done before the w=0 copy
            dummy = persist.tile([128, 128], BF16, name="dummy")
            nc.vector.memset(dummy[:], 0)
            trash = persist.tile([128, 4], BF16, name="trash")
            nc.scalar.copy(trash[:, 0:1], dummy[:, 0:1])

            imt = [persist.tile([128, 2 * P], FP8, name=f"imt{c}")
                   for c in range(K4)]
            for c in range(K4):
                nc.scalar.dma_start(imt[c][:],
                                    imT.ap()[:, c * 2 * P:(c + 1) * 2 * P])

            # all s words in one persistent tile; per-word DMAs paced with
            # ~10 words of lookahead so transfers never burst (SBUF-port
            # contention with the PE) nor starve the stream
            s_all = persist.tile([128, int(soff[-1])], FP8, name="s_all")

            def s_dma(w):
                nc.sync.dma_start(s_all[:, int(soff[w]):int(soff[w + 1])],
                                  sT.ap()[:, int(soff[w]):int(soff[w + 1])])

            for w in range(min(RING, weff)):
                s_dma(w)
            mkt = persist.tile([128, npc * IPC], F16, name="mkt")
            nc.scalar.dma_start(mkt[:], mk.ap()[:])
            eyet = persist.tile([128, 128], F16, name="eyet")
            nc.scalar.dma_start(eyet[:], eye.ap()[:])

            def s_lhsT(w):      # [128, 2, b_w] zero-padded stationary
                return s_all[:, int(soff[w]):int(soff[w + 1])].rearrange(
                    "p (c h j) -> p c h j", c=K4, h=2)

            def im_c(c):        # [128, 2, P] moving operand
                return imt[c][:].rearrange("p (h n) -> p h n", h=2)

            # --- PE warm-up (clock ramp) while DMAs stream --------------
            warm = pspool.tile([128, 512], F32, tag="ps", name="warm")
            for _ in range(40):
                nc.tensor.matmul(warm[:, 0:128], dummy[:], dummy[:],
                                 start=True, stop=True)

            # --- persistent state ---------------------------------------
            # one al tile per reduce window: an Act copy's WAR then only
            # couples to its own window's readers (whole-tile fallback deps
            # otherwise serialize copies behind unrelated reduce bursts)
            alw = {}
            for (w0, n) in wins:
                alw[w0] = persist.tile([128, n, P], F16, name=f"al{w0}")
            w2win = {}
            for (w0, n) in wins:
                for v in range(w0, w0 + n):
                    w2win[v] = (w0, v - w0)
            runmax = persist.tile([128, P], F16, name="runmax")
            maxr = persist.tile([128, IPC, weff], F16, name="maxr")
            psT = pspool.tile([128, npc * 128], F16, tag="psT", name="psT", bufs=1)
            nc.gpsimd.memset(runmax[:], 0)
            nc.gpsimd.memset(maxr[:], 0)
            # PSUM can't take a bf16 memset; zero it through an f32 view
            # (gpsimd can't touch PSUM, so this one stays on DVE)
            nc.vector.memset(psT[:].bitcast(F32), 0)

            win_end = {w0 + n - 1: (w0, n) for (w0, n) in wins}
            for w in range(weff):
                bw = int(b[w])
                ps = pspool.tile([128, 512], F32, tag="ps", name=f"ps{w}")
                lhsT = s_lhsT(w)
                for c in range(K4):
                    nc.tensor.matmul(ps[0:bw, 0:P], lhsT[:, c, :, :],
                                     im_c(c), start=(c == 0),
                                     stop=(c == K4 - 1), perf_mode=DR)
                a0, ai = w2win[w]
                nc.scalar.copy(alw[a0][0:bw, ai, :], ps[0:bw, 0:P])
                nc.vector.tensor_max(runmax[0:bw, :], runmax[0:bw, :],
                                     alw[a0][0:bw, ai, :])
                if w + RING < weff:
                    s_dma(w + RING)
                # window maxr reduces (rows >= b are untouched zeros)
                if w in win_end:
                    w0, n = win_end[w]
                    at = alw[w0]
                    for (k0, g, s) in groups:
                        b0 = int(base[k0])
                        if g == 1:
                            nc.vector.reduce_max(
                                maxr[0:bw, k0, w0:w0 + n],
                                at[0:bw, 0:n, b0:b0 + s], axis=X)
                        else:
                            # g (stratum) outer so the out keeps w innermost
                            # at stride 1 (2x_1p needs a packed last dim)
                            nc.vector.reduce_max(
                                maxr[0:bw, k0:k0 + g, w0:w0 + n],
                                at[0:bw, 0:n, b0:b0 + g * s].rearrange(
                                    "p w (g s) -> p g w s", g=g),
                                axis=X)

            # --- tail ----------------------------------------------------
            # term1[j, i] = sum_w maxr
            t1 = persist.tile([128, IPC], F32, name="t1")
            nc.vector.reduce_sum(t1[:], maxr[:], axis=X)
            # term2[i, j] = sum_r runmax via transpose + 0/1 mask matmul
            pv = psT[:].rearrange("p (t j) -> p t j", t=npc)
            for t, pc in enumerate(pcs):
                nc.tensor.transpose(pv[0:pc, t, :],
                                    runmax[:, t * 128:t * 128 + pc], eyet[:])
            rmT = persist.tile([128, npc, 128], F16, name="rmT")
            nc.scalar.copy(rmT[:], pv)
            ps2 = pspool.tile([IPC, 128], F32, tag="ps2", name="ps2", bufs=1)
            for t, pc in enumerate(pcs):
                nc.tensor.matmul(ps2[:], mkt[0:pc, t * IPC:(t + 1) * IPC],
                                 rmT[0:pc, t, :], start=(t == 0),
                                 stop=(t == npc - 1))
            # transpose term2 back to [j, i] (bf16) and add to term1
            t2sb = persist.tile([IPC, B], F16, name="t2sb")
            nc.scalar.copy(t2sb[:], ps2[:])
            psT2 = pspool.tile([B, IPC], F16, tag="psT2", name="psT2", bufs=1)
            nc.tensor.transpose(psT2[:], t2sb[:], eyet[0:IPC, 0:IPC])
            res = persist.tile([B, IPC], F32, name="res")
            nc.vector.tensor_add(res[:], t1[:], psT2[:])
            nc.sync.dma_start(out.ap()[:], res[:])

    nc.compile()
    return nc


def _get_nc(plan):
    key = (tuple(plan["strides"].tolist()), tuple(plan["cnt"].tolist()))
    if key not in _NC_CACHE:
        _NC_CACHE[key] = _build(plan)
    return _NC_CACHE[key]


def kernel(im_set, s_seq, im_len, s_len):
    im_set = np.asarray(im_set, dtype=np.float32)
    s_seq = np.asarray(s_seq, dtype=np.float32)
    plan = _plan(im_len, s_len)
    il, deal = plan["il"], plan["deal"]
    strides, P, base = plan["strides"], plan["P"], plan["base"]
    jorder, weff, cnt = plan["jorder"], plan["weff"], plan["cnt"]

    im = im_set[:, 1:, :]                    # [B, R, D]
    s = s_seq[:, 1:-2, :]                    # [B, W, D]

    # sT: concat over words of [dk, (k4, h, j<b_w)], zero-padded to b_w
    bws = plan["b"]
    s_j = s[jorder]                          # [B sorted, W, D]
    parts = []
    for w in range(weff):
        cw, bw = int(cnt[w]), int(bws[w])
        blk = np.zeros((D, bw), dtype=np.float32)
        blk[:, :cw] = s_j[:cw, w, :].T
        parts.append(blk.reshape(K4, 2, 128, bw)
                     .transpose(2, 0, 1, 3).reshape(128, K4 * 2 * bw))
    sT = np.ascontiguousarray(np.concatenate(parts, axis=1)).astype(
        ml_dtypes.float8_e4m3)

    # mk: 0/1 stratum-membership mask [p, (pchunk, i)]
    npc = (P + 127) // 128
    mkm = np.zeros((128, npc, IPC), dtype=np.float32)
    for k in range(IPC):
        for g in range(int(base[k]), int(base[k + 1])):
            mkm[g % 128, g // 128, k] = 1.0
    mkm = mkm.reshape(128, npc * IPC).astype(np.float16)
    eye = np.eye(128, dtype=np.float32).astype(np.float16)

    in_maps = []
    for c in range(NCORES):
        imp = np.zeros((P, D), dtype=np.float32)
        for k in range(IPC):
            img = deal[k, c]
            n = int(il[img])
            imp[int(base[k]):int(base[k]) + n] = im[img, :n]
        imT = (imp.T.reshape(K4, 2, 128, P).transpose(2, 0, 1, 3)
               .reshape(128, K4 * 2 * P).astype(ml_dtypes.float8_e4m3))
        in_maps.append({"sT": sT, "imT": np.ascontiguousarray(imT),
                        "mk": mkm, "eye": eye})

    nc = _get_nc(plan)
    # The accelerator sporadically reports NRT_EXEC_UNIT_UNRECOVERABLE on the
    # first execution of a freshly loaded NEFF; it recovers after a pause.
    import time
    res = None
    for attempt in range(4):
        try:
            res = run_bass_kernel_spmd(nc, in_maps,
                                       core_ids=list(range(NCORES)))
            break
        except Exception:
            if attempt == 3:
                raise
            time.sleep(15 * (attempt + 1))

    full = np.empty((B, B), dtype=np.float32)
    for c in range(NCORES):
        o = res.results[c]["out"]            # [B, IPC] rows=sorted j, cols=strata
        full[deal[:, c][:, None], jorder[None, :]] = o.T
    return full


# revision 30
# speedup vs baseline: 1.2315x; 1.0153x over previous
"""AlignmentContrastiveLoss (MrSw) TRN2 kernel — packed fp8 DoubleRow einsum.

Data-parallel over images (16 per core).  Two packings cut both PE columns
and HBM traffic:

* im packing: images are sorted by valid-region count il and dealt into 16
  strata (stratum k = rank 8k..8k+7, one image per core).  Stratum k is
  stored at a shared stride_k = max il in the stratum (+1 zero slot when
  il<49 so the reference's masked-0 candidates survive the max).  Packed
  width P = sum(strides) <= 512, so each word needs only 4 accumulating
  K=256 fp8 DoubleRow matmuls into a single PSUM bank.

* s packing: sentences are sorted by valid-word count sl; word w ships only
  its cnt_w = #{j: sl_j > w} valid columns (nested prefixes), so the
  stationary is [dk, 2, cnt_w] and sT shrinks ~40%.  Output partitions
  >= cnt_w are stale; reads are partition-limited and maxr rows are
  re-zeroed per word.

Reductions: ScalarE evacuates PSUM->SBUF bf16 ring; GpSimd+DVE split the
runmax elementwise max; DVE does per-stratum maxr reduces batched in
5-word windows; term2 (sum_r runmax per image) runs on the PE as
transpose + 0/1-mask matmul so the tail stays off DVE.
"""

import numpy as np
import ml_dtypes

import concourse.bacc as bacc
import concourse.mybir as mybir
import concourse.tile as tile
from concourse.bass_utils import run_bass_kernel_spmd

B = 128
L_IM, L_S, D = 50, 40, 1024
R = L_IM - 1     # 49
W = L_S - 3      # 37
NCORES = 8
IPC = B // NCORES            # 16
K4 = D // 256                # 4 double-row contraction chunks
FP8 = mybir.dt.float8e4
BF16 = mybir.dt.bfloat16
F16 = mybir.dt.float16
F32 = mybir.dt.float32
X = mybir.AxisListType.X
DR = mybir.MatmulPerfMode.DoubleRow

RING = 10        # s-word ring and al ring depth
WIN = 14         # max maxr reduce window (words)
NEG = -1.0e30

_NC_CACHE = {}


def _plan(im_len, s_len):
    il = (np.asarray(im_len).astype(np.int64) - 1)
    sl = (np.asarray(s_len).astype(np.int64) - 3)
    iorder = np.argsort(-il, kind="stable")          # image deal, rank-major
    deal = iorder.reshape(IPC, NCORES)               # [k, c] image ids
    smax = il[deal].max(axis=1)
    strides = np.where(smax < R, smax + 1, R).astype(np.int64)
    # pair adjacent strata (equal stride) while the P<=512 budget lasts so
    # each reduce window needs fewer DVE ops
    budget = 510 - int(strides.sum())   # 512 exactly = pow2 stride pathology
    paired = []
    for k in sorted(range(0, IPC - 1, 2),
                    key=lambda k: strides[k] - strides[k + 1]):
        cost = int(strides[k] - strides[k + 1])
        if cost <= budget:
            budget -= cost
            paired.append(k)
    for k in paired:
        strides[k + 1] = strides[k]
    groups = []                                      # (k0, n_strata, stride)
    k = 0
    while k < IPC:
        if k in paired:
            groups.append((k, 2, int(strides[k])))
            k += 2
        else:
            groups.append((k, 1, int(strides[k])))
            k += 1
    P = int(strides.sum())
    base = np.concatenate([[0], np.cumsum(strides)]).astype(np.int64)
    jorder = np.argsort(-sl, kind="stable")          # sentence perm
    slp = sl[jorder]
    weff = int(slp[0])
    cnt = np.array([(slp > w).sum() for w in range(weff)], dtype=np.int64)
    b = ((cnt + 31) // 32) * 32                      # padded stationary width
    wins = []                                        # (w0, n) within const-b runs
    s = 0
    for w in range(1, weff + 1):
        if w == weff or b[w] != b[s]:
            w0 = s
            while w0 < w:
                n = min(WIN, w - w0)
                wins.append((w0, n))
                w0 += n
            s = w
    if wins[-1][1] > 3:                              # short final window -> short tail
        w0, n = wins.pop()
        wins.extend([(w0, n - 3), (w0 + n - 3, 3)])
    if wins[0][1] > 9:          # split the first window so DVE gets reduce
        w0, n = wins[0]         # work during the early stream instead of
        wins = [(w0, 5), (w0 + 5, n - 5)] + wins[1:]   # idling behind TTs

    return dict(il=il, sl=sl, deal=deal, strides=strides, P=P, base=base,
                jorder=jorder, weff=weff, cnt=cnt, b=b, wins=wins,
                groups=groups)


def _build(plan):
    strides, base = plan["strides"], plan["base"]
    P, weff, cnt = plan["P"], plan["weff"], plan["cnt"]
    b, wins, groups = plan["b"], plan["wins"], plan["groups"]
    assert P <= 512, P
    soff = np.concatenate([[0], np.cumsum(K4 * 2 * b)]).astype(np.int64)
    pcs = [min(128, P - t * 128) for t in range((P + 127) // 128)]
    npc = len(pcs)

    nc = bacc.Bacc("TRN2", target_bir_lowering=False, debug=False,
                   num_devices=NCORES)
    # sT layout: concat over words of [dk, (k4, h, j<cnt_w)]
    sT = nc.dram_tensor("sT", [128, int(soff[-1])], FP8, kind="ExternalInput")
    # imT layout: [dk, (k4, h, packed_ir)]
    imT = nc.dram_tensor("imT", [128, K4 * 2 * P], FP8, kind="ExternalInput")
    # mk layout: [p, (pchunk, i)] 0/1 stratum-membership mask
    mk = nc.dram_tensor("mk", [128, npc * IPC], F16, kind="ExternalInput")
    eye = nc.dram_tensor("eye", [128, 128], F16, kind="ExternalInput")
    out = nc.dram_tensor("out", [B, IPC], F32, kind="ExternalOutput")

    with tile.TileContext(nc) as tc:
        with (
            tc.tile_pool(name="persist", bufs=1) as persist,
            tc.tile_pool(name="ps", bufs=5, space="PSUM") as pspool,
        ):
            # --- startup DMAs -------------------------------------------
            # ACT table pre-load first so it is done before the w=0 copy
            dummy = persist.tile([128, 128], BF16, name="dummy")
            nc.vector.memset(dummy[:], 0)
            trash = persist.tile([128, 4], BF16, name="trash")
            nc.scalar.copy(trash[:, 0:1], dummy[:, 0:1])

            imt = [persist.tile([128, 2 * P], FP8, name=f"imt{c}")
                   for c in range(K4)]
            for c in range(K4):
                nc.scalar.dma_start(imt[c][:],
                                    imT.ap()[:, c * 2 * P:(c + 1) * 2 * P])

            # all s words in one persistent tile; per-word DMAs paced with
            # ~10 words of lookahead so transfers never burst (SBUF-port
            # contention with the PE) nor starve the stream
            s_all = persist.tile([128, int(soff[-1])], FP8, name="s_all")

            def s_dma(w):
                nc.sync.dma_start(s_all[:, int(soff[w]):int(soff[w + 1])],
                                  sT.ap()[:, int(soff[w]):int(soff[w + 1])])

            for w in range(min(RING, weff)):
                s_dma(w)
            mkt = persist.tile([128, npc * IPC], F16, name="mkt")
            nc.scalar.dma_start(mkt[:], mk.ap()[:])
            eyet = persist.tile([128, 128], F16, name="eyet")
            nc.scalar.dma_start(eyet[:], eye.ap()[:])

            def s_lhsT(w):      # [128, 2, b_w] zero-padded stationary
                return s_all[:, int(soff[w]):int(soff[w + 1])].rearrange(
                    "p (c h j) -> p c h j", c=K4, h=2)

            def im_c(c):        # [128, 2, P] moving operand
                return imt[c][:].rearrange("p (h n) -> p h n", h=2)

            # --- PE warm-up (clock ramp) while DMAs stream --------------
            warm = pspool.tile([128, 512], F32, tag="ps", name="warm")
            for _ in range(40):
                nc.tensor.matmul(warm[:, 0:128], dummy[:], dummy[:],
                                 start=True, stop=True)

            # --- persistent state ---------------------------------------
            # one al tile per reduce window: an Act copy's WAR then only
            # couples to its own window's readers (whole-tile fallback deps
            # otherwise serialize copies behind unrelated reduce bursts)
            alw = {}
            for (w0, n) in wins:
                alw[w0] = persist.tile([128, n, P], F16, name=f"al{w0}")
            w2win = {}
            for (w0, n) in wins:
                for v in range(w0, w0 + n):
                    w2win[v] = (w0, v - w0)
            runmax = persist.tile([128, P], F16, name="runmax")
            maxr = persist.tile([128, IPC, weff], F16, name="maxr")
            psT = pspool.tile([128, npc * 128], F16, tag="psT", name="psT", bufs=1)
            nc.gpsimd.memset(runmax[:], 0)
            nc.gpsimd.memset(maxr[:], 0)
            # PSUM can't take a bf16 memset; zero it through an f32 view
            # (gpsimd can't touch PSUM, so this one stays on DVE)
            nc.vector.memset(psT[:].bitcast(F32), 0)

            win_end = {w0 + n - 1: (w0, n) for (w0, n) in wins}
            for w in range(weff):
                bw = int(b[w])
                ps = pspool.tile([128, 512], F32, tag="ps", name=f"ps{w}")
                lhsT = s_lhsT(w)
                for c in range(K4):
                    nc.tensor.matmul(ps[0:bw, 0:P], lhsT[:, c, :, :],
                                     im_c(c), start=(c == 0),
                                     stop=(c == K4 - 1), perf_mode=DR)
                a0, ai = w2win[w]
                nc.scalar.copy(alw[a0][0:bw, ai, :], ps[0:bw, 0:P])
                nc.vector.tensor_max(runmax[0:bw, :], runmax[0:bw, :],
                                     alw[a0][0:bw, ai, :])
                if w + RING < weff:
                    s_dma(w + RING)
                # window maxr reduces (rows >= b are untouched zeros)
                if w in win_end:
                    w0, n = win_end[w]
                    at = alw[w0]
                    for (k0, g, s) in groups:
                        b0 = int(base[k0])
                        if g == 1:
                            nc.vector.reduce_max(
                                maxr[0:bw, k0, w0:w0 + n],
                                at[0:bw, 0:n, b0:b0 + s], axis=X)
                        else:
                            # g (stratum) outer so the out keeps w innermost
                            # at stride 1 (2x_1p needs a packed last dim)
                            nc.vector.reduce_max(
                                maxr[0:bw, k0:k0 + g, w0:w0 + n],
                                at[0:bw, 0:n, b0:b0 + g * s].rearrange(
                                    "p w (g s) -> p g w s", g=g),
                                axis=X)

            # --- tail ----------------------------------------------------
            # term1[j, i] = sum_w maxr
            t1 = persist.tile([128, IPC], F32, name="t1")
            nc.vector.reduce_sum(t1[:], maxr[:], axis=X)
            # term2[i, j] = sum_r runmax via transpose + 0/1 mask matmul
            pv = psT[:].rearrange("p (t j) -> p t j", t=npc)
            for t, pc in enumerate(pcs):
                nc.tensor.transpose(pv[0:pc, t, :],
                                    runmax[:, t * 128:t * 128 + pc], eyet[:])
            rmT = persist.tile([128, npc, 128], F16, name="rmT")
            nc.scalar.copy(rmT[:], pv)
            ps2 = pspool.tile([IPC, 128], F32, tag="ps2", name="ps2", bufs=1)
            for t, pc in enumerate(pcs):
                nc.tensor.matmul(ps2[:], mkt[0:pc, t * IPC:(t + 1) * IPC],
                                 rmT[0:pc, t, :], start=(t == 0),
                                 stop=(t == npc - 1))
            # transpose term2 back to [j, i] (bf16) and add to term1
            t2sb = persist.tile([IPC, B], F16, name="t2sb")
            nc.scalar.copy(t2sb[:], ps2[:])
            psT2 = pspool.tile([B, IPC], F16, tag="psT2", name="psT2", bufs=1)
            nc.tensor.transpose(psT2[:], t2sb[:], eyet[0:IPC, 0:IPC])
            res = persist.tile([B, IPC], F32, name="res")
            nc.vector.tensor_add(res[:], t1[:], psT2[:])
            nc.sync.dma_start(out.ap()[:], res[:])

    nc.compile()
    return nc


def _get_nc(plan):
    key = (tuple(plan["strides"].tolist()), tuple(plan["cnt"].tolist()))
    if key not in _NC_CACHE:
        _NC_CACHE[key] = _build(plan)
    return _NC_CACHE[key]


def kernel(im_set, s_seq, im_len, s_len):
    im_set = np.asarray(im_set, dtype=np.float32)
    s_seq = np.asarray(s_seq, dtype=np.float32)
    plan = _plan(im_len, s_len)
    il, deal = plan["il"], plan["deal"]
    strides, P, base = plan["strides"], plan["P"], plan["base"]
    jorder, weff, cnt = plan["jorder"], plan["weff"], plan["cnt"]

    im = im_set[:, 1:, :]                    # [B, R, D]
    s = s_seq[:, 1:-2, :]                    # [B, W, D]

    # sT: concat over words of [dk, (k4, h, j<b_w)], zero-padded to b_w
    bws = plan["b"]
    s_j = s[jorder]                          # [B sorted, W, D]
    parts = []
    for w in range(weff):
        cw, bw = int(cnt[w]), int(bws[w])
        blk = np.zeros((D, bw), dtype=np.float32)
        blk[:, :cw] = s_j[:cw, w, :].T
        parts.append(blk.reshape(K4, 2, 128, bw)
                     .transpose(2, 0, 1, 3).reshape(128, K4 * 2 * bw))
    sT = np.ascontiguousarray(np.concatenate(parts, axis=1)).astype(
        ml_dtypes.float8_e4m3)

    # mk: 0/1 stratum-membership mask [p, (pchunk, i)]
    npc = (P + 127) // 128
    mkm = np.zeros((128, npc, IPC), dtype=np.float32)
    for k in range(IPC):
        for g in range(int(base[k]), int(base[k + 1])):
            mkm[g % 128, g // 128, k] = 1.0
    mkm = mkm.reshape(128, npc * IPC).astype(np.float16)
    eye = np.eye(128, dtype=np.float32).astype(np.float16)

    in_maps = []
    for c in range(NCORES):
        imp = np.zeros((P, D), dtype=np.float32)
        for k in range(IPC):
            img = deal[k, c]
            n = int(il[img])
            imp[int(base[k]):int(base[k]) + n] = im[img, :n]
        imT = (imp.T.reshape(K4, 2, 128, P).transpose(2, 0, 1, 3)
               .reshape(128, K4 * 2 * P).astype(ml_dtypes.float8_e4m3))
        in_maps.append({"sT": sT, "imT": np.ascontiguousarray(imT),
                        "mk": mkm, "eye": eye})

    nc = _get_nc(plan)
    # The accelerator sporadically reports NRT_EXEC_UNIT_UNRECOVERABLE on the
    # first execution of a freshly loaded NEFF; it recovers after a pause.
    import time
    res = None
    for attempt in range(4):
        try:
            res = run_bass_kernel_spmd(nc, in_maps,
                                       core_ids=list(range(NCORES)))
            break
        except Exception:
            if attempt == 3:
                raise
            time.sleep(15 * (attempt + 1))

    full = np.empty((B, B), dtype=np.float32)
    for c in range(NCORES):
        o = res.results[c]["out"]            # [B, IPC] rows=sorted j, cols=strata
        full[deal[:, c][:, None], jorder[None, :]] = o.T
    return full
